# revision 42
# baseline (speedup 1.0000x reference)
"""Causal self-attention (B=2,T=2048,C=1024,H=16) on 8 trn2 NeuronCores.

Sharding: core c handles batch c//4 and the 4 heads 4*(c%4)..4*(c%4)+3
(head+batch parallel). Each core computes a [2048, 1024] partial of the
output projection (bf16, contraction over its 256 y-dims); host sums the
4 partials per batch in fp32.

Compute strategy:
- qkv projections: fp8e4 DoubleRow matmuls with exact-ish hi+lo fp8
  decomposition of both x and w (hi = fp8(v), lo = fp8(v - hi)).
- rope: PE 32-block-swap matmul + DVE mul/mul/add, writing q/k as fp8
  directly; a pure-layout DMA folds [128(2h x 32e|32o), T] into
  per-head [32, 2, T] fp8 tiles for DoubleRow scores.
- scores: fp8 DoubleRow over the two 32-dim contraction subtiles;
  additive causal mask via bf16 atri/bdg rank trick; exp on Act engine
  into fp8 pair-tiles [128, 2, 1024].
- AV: transposed accumulation out[q,d] with ex as stationary: DoubleRow
  over key-tile pairs, one pass for v_hi and one for v_lo; denominator
  rides as v column 64 (ones in hi, zeros in lo).
- normalize: per-partition reciprocal + tensor_scalar, y in natural
  [q, d] layout; XBAR DMA-transpose to [d, q] bf16 for the out-proj.
- out-proj: bf16, fused og copies + batched stores.

Self-contained: hardcodes all shapes; no sibling imports.
"""
import sys

for _p in ("/opt/trn_rl_repo", "/root/.axon_site/_ro/trn_rl_repo"):
    if _p not in sys.path:
        sys.path.append(_p)

import numpy as np
import ml_dtypes

B, T, C, H = 2, 2048, 1024, 16
Dh = C // H          # 64
NCORES = 8
HPC = 4              # heads per core
NKT = T // 128       # 16 k-tiles
QW = 1024            # q-block width for scores
NQI = T // QW        # 2 q-blocks
KPQ = QW // 128      # 8 k-tiles / q-subtiles per q-block
WS = 32.0            # w_attn pre-scale so fp8 hi/lo avoids subnormals
SCALE = 1.0 / float(np.sqrt(Dh)) / (WS * WS)   # exp scale (q,k carry WS each)
NEG = -30000.0 * WS * WS                       # additive mask, pre-exp-scale

BF16 = ml_dtypes.bfloat16
FP8 = ml_dtypes.float8_e4m3

_CACHE = {}


def _build_nc():
    import concourse.mybir as mybir
    import concourse.tile as tile
    from concourse import bacc

    dt = mybir.dt
    nc = bacc.Bacc("TRN2", target_bir_lowering=False, debug=False,
                   num_devices=NCORES)

    # host layouts (see make_in_maps):
    # xt8:  [128, P(4), s(2), hl(2), T]  fp8  (cin = 256P + 128s + part)
    # wqk8: [128, P, s, hl, m(4), 128]   fp8  (m: q01,q23,k01,k23 dims)
    # wv8:  [128, P, s, hl, 256]         fp8
    # wp:   [128, c(2), 1024]            bf16
    xt8 = nc.dram_tensor("xt8", [128, 4 * 2 * 2 * T], dt.float8e4,
                         kind="ExternalInput").ap()
    wqk8 = nc.dram_tensor("wqk8", [128, 4 * 2 * 2 * 4 * 128], dt.float8e4,
                          kind="ExternalInput").ap()
    wv8 = nc.dram_tensor("wv8", [128, 4 * 2 * 2 * 256], dt.float8e4,
                         kind="ExternalInput").ap()
    wp = nc.dram_tensor("wp", [128, 2 * C], dt.bfloat16,
                        kind="ExternalInput").ap()
    cc = nc.dram_tensor("cc", [128, T], dt.bfloat16, kind="ExternalInput").ap()
    ss = nc.dram_tensor("ss", [128, T], dt.float32, kind="ExternalInput").ap()
    pswp = nc.dram_tensor("pswp", [128, 128], dt.bfloat16,
                          kind="ExternalInput").ap()
    atri = nc.dram_tensor("atri", [128, 128], dt.bfloat16,
                          kind="ExternalInput").ap()
    bdg = nc.dram_tensor("bdg", [128, 128], dt.bfloat16,
                         kind="ExternalInput").ap()
    out = nc.dram_tensor("out", [T, C], dt.bfloat16, kind="ExternalOutput").ap()

    EXP = mybir.ActivationFunctionType.Exp
    DR = mybir.MatmulPerfMode.DoubleRow

    with tile.TileContext(nc) as tc:
        with (
            tc.tile_pool(name="const", bufs=1) as constp,
            tc.tile_pool(name="qk8", bufs=1) as qk8p,
            tc.tile_pool(name="vp8", bufs=1) as vsbp,
            tc.tile_pool(name="exp", bufs=32) as expp,
            tc.tile_pool(name="ynp", bufs=2) as ynp,
            tc.tile_pool(name="ytn", bufs=1) as ytnp,
            tc.tile_pool(name="ogp", bufs=2) as ogp,
            tc.tile_pool(name="dnp", bufs=4) as dnp,
        ):
            # ---------------- constants ----------------
            wqk_sb = constp.tile([128, 4 * 2 * 2 * 4 * 128], dt.float8e4,
                                 tag="wqk", name="wqk_sb")
            wqk_v = wqk_sb[:].rearrange("p (b m e) -> p b m e", b=16, m=4)
            wqk8_v = wqk8.rearrange("p (b m e) -> p b m e", b=16, m=4)
            for m_ in (0, 2):
                nc.sync.dma_start(out=wqk_v[:, :, m_], in_=wqk8_v[:, :, m_])
            wv_sb = constp.tile([128, 4 * 2 * 2 * 256], dt.float8e4,
                                tag="wv", name="wv_sb")
            nc.sync.dma_start(out=wv_sb[:], in_=wv8)
            wp_sb = constp.tile([128, 2 * C], dt.bfloat16, tag="wp",
                                name="wp_sb")
            nc.sync.dma_start(out=wp_sb[:], in_=wp)
            cc_sb = constp.tile([128, T], dt.bfloat16, tag="cc", name="cc_sb")
            nc.sync.dma_start(out=cc_sb[:], in_=cc)
            ss_sb = constp.tile([128, T], dt.float32, tag="ss", name="ss_sb")
            nc.sync.dma_start(out=ss_sb[:], in_=ss)
            pswp_sb = constp.tile([128, 128], dt.bfloat16, tag="pswp",
                                  name="pswp_sb")
            nc.sync.dma_start(out=pswp_sb[:], in_=pswp)
            atri_sb = constp.tile([128, 128], dt.bfloat16, tag="atri",
                                  name="atri_sb")
            nc.sync.dma_start(out=atri_sb[:], in_=atri)
            bdg_sb = constp.tile([128, 128], dt.bfloat16, tag="bdg",
                                 name="bdg_sb")
            nc.sync.dma_start(out=bdg_sb[:], in_=bdg)

            # persistent fp8 q/k per-head tiles [32, 2, T] and v store
            qf = [qk8p.tile([32, 2 * T], dt.float8e4, tag=f"qf{h}",
                            name=f"qf{h}") for h in range(HPC)]
            kf = [qk8p.tile([32, 2 * T], dt.float8e4, tag=f"kf{h}",
                            name=f"kf{h}") for h in range(HPC)]
            # vsb: [128, t(16), hl(2), h(4), 65] fp8
            vsb = vsbp.tile([128, NKT * 2 * HPC * 65], dt.float8e4,
                            tag="vsb", name="vsb")
            vsb4 = vsb[:].rearrange("p (t l h e) -> p t l h e",
                                    t=NKT, l=2, h=HPC)
            # ones col 64: hi=1, lo=0
            nc.gpsimd.memset(vsb[:], 1.0)
            nc.gpsimd.memset(vsb4[:, :, 1, :, 64:65], 0.0)

            # y transposed store [128, c(2), T] bf16 for out-proj
            yTn = ytnp.tile([128, 2 * T], dt.bfloat16, tag="yTn", name="yTn")
            yTn3 = yTn[:].rearrange("p (c t) -> p c t", c=2)

            with tc.tile_pool(name="xtp", bufs=1) as xtp, \
                 tc.tile_pool(name="stg", bufs=3) as stg:
                # xt pair tiles [128, s, hl, T], split-loaded per (P, half)
                xt = [xtp.tile([128, 2 * 2 * T], dt.float8e4, tag=f"xt{P}",
                               name=f"xt{P}") for P in range(4)]
                CH = 2 * 2 * T
                for Th in range(2):
                    for qq_ in range(2):
                        for P in range(4):
                            qc = Th * 1024 + qq_ * 512
                            nc.sync.dma_start(
                                out=xt[P][:].rearrange(
                                    "p (c t) -> p c t", c=4)[:, :, qc:qc + 512],
                                in_=xt8[:, P * CH:(P + 1) * CH].rearrange(
                                    "p (c t) -> p c t", c=4)[:, :, qc:qc + 512])
                    if Th == 0:
                        for m_ in (1, 3):
                            nc.sync.dma_start(out=wqk_v[:, :, m_],
                                              in_=wqk8_v[:, :, m_])
                xt4 = [x[:].rearrange("p (s l t) -> p s l t", s=2, l=2)
                       for x in xt]
                wqk6 = wqk_sb[:].rearrange("p (P s l m e) -> p P s l m e",
                                           P=4, s=2, l=2, m=4)
                wv5 = wv_sb[:].rearrange("p (P s l e) -> p P s l e",
                                         P=4, s=2, l=2)

                qf3 = [q[:].rearrange("p (s t) -> p s t", s=2) for q in qf]
                kf3 = [k[:].rearrange("p (s t) -> p s t", s=2) for k in kf]
                vsb4a = vsb[:].rearrange("p (t l h e) -> p t l h e",
                                         t=NKT, l=2, h=HPC)

                projq = []
                workq = []

                def pop(n=1):
                    for _ in range(n):
                        if projq:
                            projq.pop(0)()
                        elif workq:
                            workq.pop(0)()

                def drain_proj():
                    while projq:
                        projq.pop(0)()

                def emit_qk_chunk(psPJ, m, quarter, Ps=range(4), xp=None):
                    cs = slice(quarter * 512, (quarter + 1) * 512)
                    if xp is None:
                        xp = psPJ.tile([128, 512], dt.float32, tag="pj",
                                       name="xp")
                    for P in Ps:
                        for i, (xl, wl) in enumerate(((0, 0), (1, 0), (0, 1))):
                            nc.tensor.matmul(
                                out=xp[:],
                                lhsT=wqk6[:, P, :, wl, m],
                                rhs=xt4[P][:, :, xl, cs],
                                start=(P == 0 and i == 0),
                                stop=(P == 3 and i == 2),
                                perf_mode=DR)
                    if max(Ps) < 3:
                        return xp
                    xsb = stg.tile([128, 512], dt.bfloat16, tag="xsb",
                                   name="xsb")
                    nc.scalar.copy(out=xsb[:], in_=xp[:])
                    xs = psPJ.tile([128, 512], dt.float32, tag="pj", name="xs")
                    nc.tensor.matmul(out=xs[:], lhsT=pswp_sb[:], rhs=xsb[:],
                                     start=True, stop=True)
                    r1 = stg.tile([128, 512], dt.bfloat16, tag="r1", name="r1")
                    nc.vector.tensor_mul(out=r1[:], in0=xsb[:], in1=cc_sb[:, cs])
                    r2 = stg.tile([128, 512], dt.bfloat16, tag="r2", name="r2")
                    nc.vector.tensor_mul(out=r2[:], in0=xs[:], in1=ss_sb[:, cs])
                    nc.vector.tensor_add(out=dst8[m][:, cs], in0=r1[:],
                                         in1=r2[:])

                def emit_fold(m, half):
                    cs = slice(half * 1024, half * 1024 + 1024)
                    for j in range(2):
                        dest = (qf if m < 2 else kf)[(m % 2) * 2 + j]
                        for s_ in range(2):
                            nc.sync.dma_start(
                                out=dest[:, s_ * T + half * 1024:
                                         s_ * T + half * 1024 + 1024],
                                in_=dst8[m][j * 64 + s_ * 32:
                                            j * 64 + (s_ + 1) * 32, cs])

                def emit_v_unit(psPJ, rt):
                    def unit():
                        vp = psPJ.tile([128, 256], dt.float32, tag="pj",
                                       name="vp")
                        ts = slice(rt * 128, (rt + 1) * 128)
                        for P in range(4):
                            for i, (xl, wl) in enumerate(((0, 0), (1, 0),
                                                          (0, 1))):
                                nc.tensor.matmul(
                                    out=vp[:],
                                    lhsT=xt4[P][:, :, xl, ts],
                                    rhs=wv5[:, P, :, wl],
                                    start=(P == 0 and i == 0),
                                    stop=(P == 3 and i == 2),
                                    perf_mode=DR)
                        vp3 = vp[:].rearrange("p (h e) -> p h e", h=HPC)
                        nc.vector.tensor_copy(out=vsb4a[:, rt, 0, :, 0:64],
                                              in_=vp3)
                        nc.vector.tensor_sub(out=vsb4a[:, rt, 1, :, 0:64],
                                             in0=vp3,
                                             in1=vsb4a[:, rt, 0, :, 0:64])
                    return unit

                def emit_scores(psS, h, qi, t):
                    p = t - KPQ * qi
                    j0 = 128 * p if p > 0 else 0
                    mask_bank = j0 // 512 if p >= 0 else -1
                    sc = psS.tile([128, QW], dt.float32, tag="sc", name="sc")
                    for bk in range(2):
                        lo, hi = bk * 512, (bk + 1) * 512
                        lo = max(lo, j0)
                        if lo >= hi:
                            continue
                        nc.tensor.matmul(
                            out=sc[:, lo:hi],
                            lhsT=kf3[h][:, :, t * 128:(t + 1) * 128],
                            rhs=qf3[h][:, :, qi * QW + lo:qi * QW + hi],
                            start=True, stop=(bk != mask_bank),
                            perf_mode=DR)
                    if p >= 0:
                        nc.tensor.matmul(
                            out=sc[:, j0:j0 + 128],
                            lhsT=atri_sb[:], rhs=bdg_sb[:],
                            start=False, stop=True)
                    return sc, j0

                def emit_av_unit(psY, qi, h, qt, exs, sink):
                    def unit():
                        qtg = qi * KPQ + qt
                        cs = slice(qt * 128, (qt + 1) * 128)
                        yt = psY.tile([128, 65], dt.float32, tag="yt",
                                      name="yt")
                        nfull = (qtg + 1) // 2
                        single = (qtg % 2 == 0)
                        for u in range(nfull):
                            ex3 = exs[u][:].rearrange("p (s q) -> p s q", s=2)
                            for li in range(2):
                                is_last = (not single and u == nfull - 1
                                           and li == 1)
                                nc.tensor.matmul(
                                    out=yt[:],
                                    lhsT=ex3[:, :, cs],
                                    rhs=vsb4a[:, 2 * u:2 * u + 2, li, h],
                                    start=(u == 0 and li == 0), stop=is_last,
                                    perf_mode=DR)
                        if single:
                            ts_ = qtg
                            ex2 = exs[ts_ // 2][:].rearrange(
                                "p (s q) -> p s q", s=2)[:, ts_ % 2]
                            for li in range(2):
                                nc.tensor.matmul(
                                    out=yt[:],
                                    lhsT=ex2[:, cs],
                                    rhs=vsb4a[:, ts_, li, h],
                                    start=(qtg == 0 and li == 0),
                                    stop=(li == 1))
                        sink.append(yt)
                    return unit

                def emit_norm_unit(qi, h, qt, sink, yns):
                    def unit():
                        yt = sink.pop(0)
                        dn = dnp.tile([128, 1], dt.float32, tag="dn",
                                      name="dn")
                        nc.vector.reciprocal(out=dn[:], in_=yt[:, 64:65])
                        nc.vector.tensor_scalar(
                            out=yns[qt][:, h * 64:(h + 1) * 64],
                            in0=yt[:, 0:64],
                            scalar1=dn[:], scalar2=1.0 / WS,
                            op0=mybir.AluOpType.mult,
                            op1=mybir.AluOpType.mult)
                    return unit

                def emit_block2(psS, psY, qi, hA, hB, yns, flush=None,
                                chain=False):
                    tmax = KPQ * qi + KPQ - 1
                    exsA, exsB = [], []
                    sinkA, sinkB = [], []

                    def flush_qt(qt):
                        emit_av_unit(psY, qi, hA, qt, exsA, sinkA)()
                        emit_norm_unit(qi, hA, qt, sinkA, yns)()
                        emit_av_unit(psY, qi, hB, qt, exsB, sinkB)()
                        emit_norm_unit(qi, hB, qt, sinkB, yns)()
                        if flush is not None:
                            flush(qt)

                    for t in range(tmax + 1):
                        scA, j0 = emit_scores(psS, hA, qi, t)
                        scB, _ = emit_scores(psS, hB, qi, t)
                        if t % 2 == 0:
                            exsA.append(expp.tile([128, 2 * QW], dt.float8e4,
                                                  tag="ex", name="exA"))
                            exsB.append(expp.tile([128, 2 * QW], dt.float8e4,
                                                  tag="ex", name="exB"))
                        nc.scalar.activation(
                            out=exsA[-1][:, (t % 2) * QW + j0:
                                         (t % 2) * QW + QW],
                            in_=scA[:, j0:QW], func=EXP, scale=SCALE)
                        nc.scalar.activation(
                            out=exsB[-1][:, (t % 2) * QW + j0:
                                         (t % 2) * QW + QW],
                            in_=scB[:, j0:QW], func=EXP, scale=SCALE)
                        if flush is not None:
                            kq = t - (KPQ * qi) - 3
                            if 0 <= kq < KPQ:
                                flush_qt(kq)
                            pop(4)
                        else:
                            pop(2)
                    if flush is not None:
                        for kq in range(max(0, tmax - KPQ * qi - 2), KPQ):
                            flush_qt(kq)
                        return
                    if chain:
                        sinkA, sinkB = [], []
                        for qt in range(KPQ):
                            workq.append(
                                emit_av_unit(psY, qi, hA, qt, exsA, sinkA))
                            workq.append(
                                emit_norm_unit(qi, hA, qt, sinkA, yns))
                            workq.append(
                                emit_av_unit(psY, qi, hB, qt, exsB, sinkB))
                            workq.append(
                                emit_norm_unit(qi, hB, qt, sinkB, yns))
                            workq.append(emit_transpose_unit(qi, qt, yns))
                            if qt % 2 == 1:
                                og2 = ogp.tile([128, 2 * C], dt.bfloat16,
                                               tag="og", name="og2")
                                rt0 = qi * KPQ + qt - 1
                                for half in range(2):
                                    workq.append(
                                        emit_outproj_unit(psO, rt0 + half,
                                                          og2, half))
                                workq.append(emit_store_unit(og2, rt0))
                        return
                    for h, exs in ((hA, exsA), (hB, exsB)):
                        sink = []
                        for qt in range(KPQ):
                            workq.append(
                                emit_av_unit(psY, qi, h, qt, exs, sink))
                            if qt >= 1:
                                workq.append(
                                    emit_norm_unit(qi, h, qt - 1, sink, yns))
                        workq.append(
                            emit_norm_unit(qi, h, KPQ - 1, sink, yns))

                def emit_transpose_unit(qi, qt, yns):
                    def unit():
                        for c_ in range(2):
                            nc.sync.dma_start_transpose(
                                out=yTn3[:, c_, (qi * KPQ + qt) * 128:
                                         (qi * KPQ + qt + 1) * 128],
                                in_=yns[qt][:, c_ * 128:(c_ + 1) * 128])
                    return unit

                def emit_outproj_unit(psO, rt, og2, half):
                    def unit():
                        for ct in range(2):
                            op = psO.tile([128, 512], dt.float32, tag="op",
                                          name="op")
                            for c_ in range(2):
                                nc.tensor.matmul(
                                    out=op[:],
                                    lhsT=yTn3[:, c_, rt * 128:(rt + 1) * 128],
                                    rhs=wp_sb[:]
                                        .rearrange("p (c e) -> p c e", c=2)
                                        [:, c_, ct * 512:(ct + 1) * 512],
                                    start=(c_ == 0), stop=(c_ == 1))
                            nc.vector.tensor_copy(
                                out=og2[:].rearrange("p (r e) -> p r e", r=2)
                                    [:, half, ct * 512:(ct + 1) * 512],
                                in_=op[:])
                    return unit

                def emit_store_unit(og2, rt0):
                    def unit():
                        nc.sync.dma_start(
                            out=out[rt0 * 128:(rt0 + 2) * 128, :]
                                .rearrange("(r p) e -> p r e", r=2),
                            in_=og2[:].rearrange("p (r e) -> p r e", r=2))
                    return unit

                # m: 0=q(h0,h1) 1=q(h2,h3) 2=k(h0,h1) 3=k(h2,h3)
                dst8 = [stg.tile([128, T], dt.float8e4, tag=f"d8{m}",
                                 name=f"d8{m}", bufs=1) for m in range(4)]

                with (
                    tc.tile_pool(name="psS", bufs=2, space="PSUM",
                                 side="left") as psS,
                    tc.tile_pool(name="psY", bufs=2, space="PSUM",
                                 side="left") as psY,
                ):
                    with tc.tile_pool(name="psPJ", bufs=2, space="PSUM",
                                      side="right") as psPJ:
                        # q,k halves for heads 0/1 upfront (enough for qi0)
                        for m in (0, 2):
                            for quarter in (0, 1):
                                emit_qk_chunk(psPJ, m, quarter)
                            emit_fold(m, 0)
                        # second halves of m0/m2 as fillers for block (0,0,1)
                        for m in (0, 2):
                            for quarter in (2, 3):
                                projq.append(
                                    (lambda mm, qq:
                                     lambda: emit_qk_chunk(psPJ, mm, qq))
                                    (m, quarter))
                            projq.append(
                                (lambda mm: lambda: emit_fold(mm, 1))(m))

                        yns_all = {}
                        for qi in range(NQI):
                            yns_all[qi] = [
                                ynp.tile([128, 256], dt.bfloat16,
                                         tag=f"yn{q}", name=f"yn{q}")
                                for q in range(KPQ)]

                        def emit_qi_tail(qi):
                            for qt in range(KPQ):
                                workq.append(
                                    emit_transpose_unit(qi, qt, yns_all[qi]))
                            for rp in range(KPQ // 2):
                                rt0 = qi * KPQ + rp * 2
                                og2 = ogp.tile([128, 2 * C], dt.bfloat16,
                                               tag="og", name="og2")
                                for half in range(2):
                                    workq.append(
                                        emit_outproj_unit(psO, rt0 + half,
                                                          og2, half))
                                workq.append(emit_store_unit(og2, rt0))

                        emit_block2(psS, psY, 0, 0, 1, yns_all[0])
                        drain_proj()
                        def qk_split_units(mm, qq):
                            hold = {}

                            def unit_a():
                                hold["xp"] = emit_qk_chunk(
                                    psPJ, mm, qq, Ps=(0, 1))

                            def unit_b():
                                emit_qk_chunk(psPJ, mm, qq, Ps=(2, 3),
                                              xp=hold["xp"])
                            return unit_a, unit_b

                        for m in (1, 3):
                            for quarter in range(4):
                                ua, ub = qk_split_units(m, quarter)
                                projq.append(ua)
                                projq.append(ub)
                            for half in range(2):
                                projq.append(
                                    (lambda mm, hh:
                                     lambda: emit_fold(mm, hh))(m, half))
                        for rt in range(NKT):
                            projq.append(
                                (lambda r: lambda: emit_v_unit(psPJ, r)())(rt))
                        emit_block2(psS, psY, 1, 0, 1, yns_all[1])
                        drain_proj()
                    with tc.tile_pool(name="psO", bufs=2, space="PSUM",
                                      side="right") as psO:
                        emit_block2(psS, psY, 0, 2, 3, yns_all[0])
                        emit_qi_tail(0)
                        emit_block2(psS, psY, 1, 2, 3, yns_all[1])
                        emit_qi_tail(1)
                        while projq or workq:
                            pop()
    nc.compile()
    return nc


def get_nc():
    if "nc" not in _CACHE:
        _CACHE["nc"] = _build_nc()
    return _CACHE["nc"]


def _hilo(a):
    hi = a.astype(FP8)
    lo = (a - hi.astype(np.float32)).astype(FP8)
    return hi, lo


def make_in_maps(x, w_attn, w_proj, freqs_cos, freqs_sin):
    x = np.asarray(x, dtype=np.float32)
    w_attn = np.asarray(w_attn, dtype=np.float32)
    w_proj = np.asarray(w_proj, dtype=np.float32)
    freqs_cos = np.asarray(freqs_cos, dtype=np.float32)
    freqs_sin = np.asarray(freqs_sin, dtype=np.float32)

    # rope tables: per 64-d head block = [32 even | 32 odd], 2 heads/chunk
    cos_t = freqs_cos.T
    sin_t = freqs_sin.T
    cc = np.concatenate([cos_t] * 4, axis=0).astype(BF16)
    ss = np.concatenate([-sin_t, sin_t, -sin_t, sin_t], axis=0)\
        .astype(np.float32)

    pswp = np.zeros((128, 128), dtype=np.float32)
    for i in range(128):
        pswp[i, (i // 32 ^ 1) * 32 + i % 32] = 1.0
    pswp = pswp.astype(BF16)

    atri = np.triu(np.ones((128, 128), dtype=np.float32), k=1).astype(BF16)
    bdg = (NEG * np.eye(128, dtype=np.float32)).astype(BF16)

    perm = np.concatenate([np.arange(0, Dh, 2), np.arange(1, Dh, 2)])

    in_maps = []
    for c in range(NCORES):
        b = c // 4
        h0 = HPC * (c % 4)
        # x^T for this batch: [1024 cin, T], hi/lo, [128, P, s, hl, T]
        xt = np.ascontiguousarray(x[b].reshape(T, C).T)
        xhi, xlo = _hilo(xt)
        x5 = np.stack([xhi.reshape(4, 2, 128, T), xlo.reshape(4, 2, 128, T)],
                      axis=2)                       # [P, s, hl, 128, T]
        xt8 = np.ascontiguousarray(x5.transpose(3, 0, 1, 2, 4)
                                   .reshape(128, -1))

        # wqk columns: m-chunks (q h0h1, q h2h3, k h0h1, k h2h3), each
        # 128 cols = 2 heads x [32 even | 32 odd]
        cols = []
        for off in (0, C):
            for j0 in (0, 2):
                blk = [off + (h0 + j0 + j) * Dh + perm for j in range(2)]
                cols.append(np.concatenate(blk))
        wqk_c = w_attn[:, np.stack(cols, 0).reshape(-1)] * WS  # [1024, 512]
        whi, wlo = _hilo(wqk_c)
        # [P, s, 128, hl, m, 128] -> [128, P, s, hl, m, 128]
        w6 = np.stack([whi.reshape(4, 2, 128, 4, 128),
                       wlo.reshape(4, 2, 128, 4, 128)], axis=3)
        wqk8 = np.ascontiguousarray(w6.transpose(2, 0, 1, 3, 4, 5)
                                    .reshape(128, -1))

        wv_c = w_attn[:, 2 * C + h0 * Dh: 2 * C + (h0 + HPC) * Dh] * WS
        vhi, vlo = _hilo(wv_c)
        v5 = np.stack([vhi.reshape(4, 2, 128, 256),
                       vlo.reshape(4, 2, 128, 256)], axis=3)
        wv8 = np.ascontiguousarray(v5.transpose(2, 0, 1, 3, 4)
                                   .reshape(128, -1))

        wp_c = w_proj[h0 * Dh:(h0 + HPC) * Dh, :]      # [256, 1024]
        wp8 = np.ascontiguousarray(
            wp_c.reshape(2, 128, C).transpose(1, 0, 2).reshape(128, -1))\
            .astype(BF16)

        in_maps.append({
            "xt8": xt8, "wqk8": wqk8, "wv8": wv8, "wp": wp8,
            "cc": cc, "ss": ss, "pswp": pswp, "atri": atri, "bdg": bdg,
        })
    return in_maps


def kernel(x, w_attn, w_proj, freqs_cos, freqs_sin):
    from concourse import bass_utils

    nc = get_nc()
    in_maps = make_in_maps(x, w_attn, w_proj, freqs_cos, freqs_sin)
    res = bass_utils.run_bass_kernel_spmd(
        nc, in_maps, core_ids=list(range(NCORES)), trace=False)
    outs = []
    for b in range(B):
        acc = res.results[4 * b]["out"].astype(np.float32)
        for j in range(1, 4):
            acc += res.results[4 * b + j]["out"].astype(np.float32)
        outs.append(acc)
    return np.stack(outs, 0)


# revision 45
# speedup vs baseline: 1.0047x; 1.0047x over previous
"""Causal self-attention (B=2,T=2048,C=1024,H=16) on 8 trn2 NeuronCores.

Sharding: core c handles batch c//4 and the 4 heads 4*(c%4)..4*(c%4)+3
(head+batch parallel). Each core computes a [2048, 1024] partial of the
output projection (bf16, contraction over its 256 y-dims); host sums the
4 partials per batch in fp32.

Compute strategy:
- qkv projections: fp8e4 DoubleRow matmuls with exact-ish hi+lo fp8
  decomposition of both x and w (hi = fp8(v), lo = fp8(v - hi)).
- rope: PE 32-block-swap matmul + DVE mul/mul/add, writing q/k as fp8
  directly; a pure-layout DMA folds [128(2h x 32e|32o), T] into
  per-head [32, 2, T] fp8 tiles for DoubleRow scores.
- scores: fp8 DoubleRow over the two 32-dim contraction subtiles;
  additive causal mask via bf16 atri/bdg rank trick; exp on Act engine
  into fp8 pair-tiles [128, 2, 1024].
- AV: transposed accumulation out[q,d] with ex as stationary: DoubleRow
  over key-tile pairs, one pass for v_hi and one for v_lo; denominator
  rides as v column 64 (ones in hi, zeros in lo).
- normalize: per-partition reciprocal + tensor_scalar, y in natural
  [q, d] layout; XBAR DMA-transpose to [d, q] bf16 for the out-proj.
- out-proj: bf16, fused og copies + batched stores.

Self-contained: hardcodes all shapes; no sibling imports.
"""
import sys

for _p in ("/opt/trn_rl_repo", "/root/.axon_site/_ro/trn_rl_repo"):
    if _p not in sys.path:
        sys.path.append(_p)

import numpy as np
import ml_dtypes

B, T, C, H = 2, 2048, 1024, 16
Dh = C // H          # 64
NCORES = 8
HPC = 4              # heads per core
NKT = T // 128       # 16 k-tiles
QW = 1024            # q-block width for scores
NQI = T // QW        # 2 q-blocks
KPQ = QW // 128      # 8 k-tiles / q-subtiles per q-block
WS = 32.0            # w_attn pre-scale so fp8 hi/lo avoids subnormals
SCALE = 1.0 / float(np.sqrt(Dh)) / (WS * WS)   # exp scale (q,k carry WS each)
NEG = -30000.0 * WS * WS                       # additive mask, pre-exp-scale

BF16 = ml_dtypes.bfloat16
FP8 = ml_dtypes.float8_e4m3

_CACHE = {}


def _build_nc():
    import concourse.mybir as mybir
    import concourse.tile as tile
    from concourse import bacc

    dt = mybir.dt
    nc = bacc.Bacc("TRN2", target_bir_lowering=False, debug=False,
                   num_devices=NCORES)

    # host layouts (see make_in_maps):
    # xt8:  [128, P(4), s(2), hl(2), T]  fp8  (cin = 256P + 128s + part)
    # wqk8: [128, P, s, hl, m(4), 128]   fp8  (m: q01,q23,k01,k23 dims)
    # wv8:  [128, P, s, hl, 256]         fp8
    # wp:   [128, c(2), 1024]            bf16
    xt8 = nc.dram_tensor("xt8", [128, 4 * 2 * 2 * T], dt.float8e4,
                         kind="ExternalInput").ap()
    wqk8 = nc.dram_tensor("wqk8", [128, 4 * 2 * 2 * 4 * 128], dt.float8e4,
                          kind="ExternalInput").ap()
    wv8 = nc.dram_tensor("wv8", [128, 4 * 2 * 2 * 256], dt.float8e4,
                         kind="ExternalInput").ap()
    wp = nc.dram_tensor("wp", [128, 2 * C], dt.bfloat16,
                        kind="ExternalInput").ap()
    cc = nc.dram_tensor("cc", [128, T], dt.bfloat16, kind="ExternalInput").ap()
    ss = nc.dram_tensor("ss", [128, T], dt.float32, kind="ExternalInput").ap()
    pswp = nc.dram_tensor("pswp", [128, 128], dt.bfloat16,
                          kind="ExternalInput").ap()
    atri = nc.dram_tensor("atri", [128, 128], dt.bfloat16,
                          kind="ExternalInput").ap()
    bdg = nc.dram_tensor("bdg", [128, 128], dt.bfloat16,
                         kind="ExternalInput").ap()
    out = nc.dram_tensor("out", [T, C], dt.bfloat16, kind="ExternalOutput").ap()

    EXP = mybir.ActivationFunctionType.Exp
    DR = mybir.MatmulPerfMode.DoubleRow

    with tile.TileContext(nc) as tc:
        with (
            tc.tile_pool(name="const", bufs=1) as constp,
            tc.tile_pool(name="qk8", bufs=1) as qk8p,
            tc.tile_pool(name="vp8", bufs=1) as vsbp,
            tc.tile_pool(name="exp", bufs=32) as expp,
            tc.tile_pool(name="ynp", bufs=2) as ynp,
            tc.tile_pool(name="ytn", bufs=1) as ytnp,
            tc.tile_pool(name="ogp", bufs=2) as ogp,
            tc.tile_pool(name="dnp", bufs=4) as dnp,
        ):
            # ---------------- constants ----------------
            wqk_sb = constp.tile([128, 4 * 2 * 2 * 4 * 128], dt.float8e4,
                                 tag="wqk", name="wqk_sb")
            wqk_v = wqk_sb[:].rearrange("p (b m e) -> p b m e", b=16, m=4)
            wqk8_v = wqk8.rearrange("p (b m e) -> p b m e", b=16, m=4)
            for m_ in (0, 2):
                nc.sync.dma_start(out=wqk_v[:, :, m_], in_=wqk8_v[:, :, m_])
            wv_sb = constp.tile([128, 4 * 2 * 2 * 256], dt.float8e4,
                                tag="wv", name="wv_sb")
            nc.sync.dma_start(out=wv_sb[:], in_=wv8)
            wp_sb = constp.tile([128, 2 * C], dt.bfloat16, tag="wp",
                                name="wp_sb")
            nc.sync.dma_start(out=wp_sb[:], in_=wp)
            cc_sb = constp.tile([128, T], dt.bfloat16, tag="cc", name="cc_sb")
            nc.sync.dma_start(out=cc_sb[:], in_=cc)
            ss_sb = constp.tile([128, T], dt.float32, tag="ss", name="ss_sb")
            nc.sync.dma_start(out=ss_sb[:], in_=ss)
            pswp_sb = constp.tile([128, 128], dt.bfloat16, tag="pswp",
                                  name="pswp_sb")
            nc.sync.dma_start(out=pswp_sb[:], in_=pswp)
            atri_sb = constp.tile([128, 128], dt.bfloat16, tag="atri",
                                  name="atri_sb")
            nc.sync.dma_start(out=atri_sb[:], in_=atri)
            bdg_sb = constp.tile([128, 128], dt.bfloat16, tag="bdg",
                                 name="bdg_sb")
            nc.sync.dma_start(out=bdg_sb[:], in_=bdg)

            # persistent fp8 q/k per-head tiles [32, 2, T] and v store
            qf = [qk8p.tile([32, 2 * T], dt.float8e4, tag=f"qf{h}",
                            name=f"qf{h}") for h in range(HPC)]
            kf = [qk8p.tile([32, 2 * T], dt.float8e4, tag=f"kf{h}",
                            name=f"kf{h}") for h in range(HPC)]
            # vsb: [128, t(16), hl(2), h(4), 65] fp8
            vsb = vsbp.tile([128, NKT * 2 * HPC * 65], dt.float8e4,
                            tag="vsb", name="vsb")
            vsb4 = vsb[:].rearrange("p (t l h e) -> p t l h e",
                                    t=NKT, l=2, h=HPC)
            # ones col 64: hi=1, lo=0
            nc.gpsimd.memset(vsb[:], 1.0)
            nc.gpsimd.memset(vsb4[:, :, 1, :, 64:65], 0.0)

            # y transposed store [128, c(2), T] bf16 for out-proj
            yTn = ytnp.tile([128, 2 * T], dt.bfloat16, tag="yTn", name="yTn")
            yTn3 = yTn[:].rearrange("p (c t) -> p c t", c=2)

            with tc.tile_pool(name="xtp", bufs=1) as xtp, \
                 tc.tile_pool(name="stg", bufs=3) as stg:
                # xt pair tiles [128, s, hl, T], split-loaded per (P, half)
                xt = [xtp.tile([128, 2 * 2 * T], dt.float8e4, tag=f"xt{P}",
                               name=f"xt{P}") for P in range(4)]
                CH = 2 * 2 * T
                for Th in range(2):
                    for qq_ in range(2):
                        for P in range(4):
                            qc = Th * 1024 + qq_ * 512
                            nc.sync.dma_start(
                                out=xt[P][:].rearrange(
                                    "p (c t) -> p c t", c=4)[:, :, qc:qc + 512],
                                in_=xt8[:, P * CH:(P + 1) * CH].rearrange(
                                    "p (c t) -> p c t", c=4)[:, :, qc:qc + 512])
                    if Th == 0:
                        for m_ in (1, 3):
                            nc.sync.dma_start(out=wqk_v[:, :, m_],
                                              in_=wqk8_v[:, :, m_])
                xt4 = [x[:].rearrange("p (s l t) -> p s l t", s=2, l=2)
                       for x in xt]
                wqk6 = wqk_sb[:].rearrange("p (P s l m e) -> p P s l m e",
                                           P=4, s=2, l=2, m=4)
                wv5 = wv_sb[:].rearrange("p (P s l e) -> p P s l e",
                                         P=4, s=2, l=2)

                qf3 = [q[:].rearrange("p (s t) -> p s t", s=2) for q in qf]
                kf3 = [k[:].rearrange("p (s t) -> p s t", s=2) for k in kf]
                vsb4a = vsb[:].rearrange("p (t l h e) -> p t l h e",
                                         t=NKT, l=2, h=HPC)

                projq = []
                workq = []

                def pop(n=1):
                    for _ in range(n):
                        if projq:
                            projq.pop(0)()
                        elif workq:
                            workq.pop(0)()

                def drain_proj():
                    while projq:
                        projq.pop(0)()

                def emit_qk_chunk(psPJ, m, quarter, Ps=range(4), xp=None):
                    cs = slice(quarter * 512, (quarter + 1) * 512)
                    if xp is None:
                        xp = psPJ.tile([128, 512], dt.float32, tag="pj",
                                       name="xp")
                    for P in Ps:
                        for i, (xl, wl) in enumerate(((0, 0), (1, 0), (0, 1))):
                            nc.tensor.matmul(
                                out=xp[:],
                                lhsT=wqk6[:, P, :, wl, m],
                                rhs=xt4[P][:, :, xl, cs],
                                start=(P == 0 and i == 0),
                                stop=(P == 3 and i == 2),
                                perf_mode=DR)
                    if max(Ps) < 3:
                        return xp
                    xsb = stg.tile([128, 512], dt.bfloat16, tag="xsb",
                                   name="xsb")
                    nc.scalar.copy(out=xsb[:], in_=xp[:])
                    xs = psPJ.tile([128, 512], dt.float32, tag="pj", name="xs")
                    nc.tensor.matmul(out=xs[:], lhsT=pswp_sb[:], rhs=xsb[:],
                                     start=True, stop=True)
                    r1 = stg.tile([128, 512], dt.bfloat16, tag="r1", name="r1")
                    nc.vector.tensor_mul(out=r1[:], in0=xsb[:], in1=cc_sb[:, cs])
                    r2 = stg.tile([128, 512], dt.bfloat16, tag="r2", name="r2")
                    nc.vector.tensor_mul(out=r2[:], in0=xs[:], in1=ss_sb[:, cs])
                    nc.vector.tensor_add(out=dst8[m][:, cs], in0=r1[:],
                                         in1=r2[:])

                def emit_fold(m, half):
                    cs = slice(half * 1024, half * 1024 + 1024)
                    for j in range(2):
                        dest = (qf if m < 2 else kf)[(m % 2) * 2 + j]
                        for s_ in range(2):
                            nc.sync.dma_start(
                                out=dest[:, s_ * T + half * 1024:
                                         s_ * T + half * 1024 + 1024],
                                in_=dst8[m][j * 64 + s_ * 32:
                                            j * 64 + (s_ + 1) * 32, cs])

                def emit_v_unit(psPJ, rt):
                    def unit():
                        vp = psPJ.tile([128, 256], dt.float32, tag="pj",
                                       name="vp")
                        ts = slice(rt * 128, (rt + 1) * 128)
                        for P in range(4):
                            for i, (xl, wl) in enumerate(((0, 0), (1, 0),
                                                          (0, 1))):
                                nc.tensor.matmul(
                                    out=vp[:],
                                    lhsT=xt4[P][:, :, xl, ts],
                                    rhs=wv5[:, P, :, wl],
                                    start=(P == 0 and i == 0),
                                    stop=(P == 3 and i == 2),
                                    perf_mode=DR)
                        vp3 = vp[:].rearrange("p (h e) -> p h e", h=HPC)
                        nc.vector.tensor_copy(out=vsb4a[:, rt, 0, :, 0:64],
                                              in_=vp3)
                        nc.vector.tensor_sub(out=vsb4a[:, rt, 1, :, 0:64],
                                             in0=vp3,
                                             in1=vsb4a[:, rt, 0, :, 0:64])
                    return unit

                def emit_scores(psS, h, qi, t):
                    p = t - KPQ * qi
                    j0 = 128 * p if p > 0 else 0
                    mask_bank = j0 // 512 if p >= 0 else -1
                    sc = psS.tile([128, QW], dt.float32, tag="sc", name="sc")
                    for bk in range(2):
                        lo, hi = bk * 512, (bk + 1) * 512
                        lo = max(lo, j0)
                        if lo >= hi:
                            continue
                        nc.tensor.matmul(
                            out=sc[:, lo:hi],
                            lhsT=kf3[h][:, :, t * 128:(t + 1) * 128],
                            rhs=qf3[h][:, :, qi * QW + lo:qi * QW + hi],
                            start=True, stop=(bk != mask_bank),
                            perf_mode=DR)
                    if p >= 0:
                        nc.tensor.matmul(
                            out=sc[:, j0:j0 + 128],
                            lhsT=atri_sb[:], rhs=bdg_sb[:],
                            start=False, stop=True)
                    return sc, j0

                def emit_av_unit(psY, qi, h, qt, exs, sink):
                    def unit():
                        qtg = qi * KPQ + qt
                        cs = slice(qt * 128, (qt + 1) * 128)
                        yt = psY.tile([128, 65], dt.float32, tag="yt",
                                      name="yt")
                        nfull = (qtg + 1) // 2
                        single = (qtg % 2 == 0)
                        for u in range(nfull):
                            ex3 = exs[u][:].rearrange("p (s q) -> p s q", s=2)
                            for li in range(2):
                                is_last = (not single and u == nfull - 1
                                           and li == 1)
                                nc.tensor.matmul(
                                    out=yt[:],
                                    lhsT=ex3[:, :, cs],
                                    rhs=vsb4a[:, 2 * u:2 * u + 2, li, h],
                                    start=(u == 0 and li == 0), stop=is_last,
                                    perf_mode=DR)
                        if single:
                            ts_ = qtg
                            ex2 = exs[ts_ // 2][:].rearrange(
                                "p (s q) -> p s q", s=2)[:, ts_ % 2]
                            for li in range(2):
                                nc.tensor.matmul(
                                    out=yt[:],
                                    lhsT=ex2[:, cs],
                                    rhs=vsb4a[:, ts_, li, h],
                                    start=(qtg == 0 and li == 0),
                                    stop=(li == 1))
                        sink.append(yt)
                    return unit

                def emit_norm_unit(qi, h, qt, sink, yns):
                    def unit():
                        yt = sink.pop(0)
                        dn = dnp.tile([128, 1], dt.float32, tag="dn",
                                      name="dn")
                        nc.vector.reciprocal(out=dn[:], in_=yt[:, 64:65])
                        nc.vector.tensor_scalar(
                            out=yns[qt][:, h * 64:(h + 1) * 64],
                            in0=yt[:, 0:64],
                            scalar1=dn[:], scalar2=1.0 / WS,
                            op0=mybir.AluOpType.mult,
                            op1=mybir.AluOpType.mult)
                    return unit

                def emit_block2(psS, psY, qi, hA, hB, yns, flush=None,
                                chain=False):
                    tmax = KPQ * qi + KPQ - 1
                    exsA, exsB = [], []
                    sinkA, sinkB = [], []

                    def flush_qt(qt):
                        emit_av_unit(psY, qi, hA, qt, exsA, sinkA)()
                        emit_norm_unit(qi, hA, qt, sinkA, yns)()
                        emit_av_unit(psY, qi, hB, qt, exsB, sinkB)()
                        emit_norm_unit(qi, hB, qt, sinkB, yns)()
                        if flush is not None:
                            flush(qt)

                    for t in range(tmax + 1):
                        scA, j0 = emit_scores(psS, hA, qi, t)
                        scB, _ = emit_scores(psS, hB, qi, t)
                        if t % 2 == 0:
                            exsA.append(expp.tile([128, 2 * QW], dt.float8e4,
                                                  tag="ex", name="exA"))
                            exsB.append(expp.tile([128, 2 * QW], dt.float8e4,
                                                  tag="ex", name="exB"))
                        nc.scalar.activation(
                            out=exsA[-1][:, (t % 2) * QW + j0:
                                         (t % 2) * QW + QW],
                            in_=scA[:, j0:QW], func=EXP, scale=SCALE)
                        nc.scalar.activation(
                            out=exsB[-1][:, (t % 2) * QW + j0:
                                         (t % 2) * QW + QW],
                            in_=scB[:, j0:QW], func=EXP, scale=SCALE)
                        if flush is not None:
                            kq = t - (KPQ * qi) - 3
                            if 0 <= kq < KPQ:
                                flush_qt(kq)
                            pop(1)
                        else:
                            pop(2)
                    if flush is not None:
                        for kq in range(max(0, tmax - KPQ * qi - 2), KPQ):
                            flush_qt(kq)
                        return
                    if chain:
                        sinkA, sinkB = [], []
                        for qt in range(KPQ):
                            workq.append(
                                emit_av_unit(psY, qi, hA, qt, exsA, sinkA))
                            workq.append(
                                emit_norm_unit(qi, hA, qt, sinkA, yns))
                            workq.append(
                                emit_av_unit(psY, qi, hB, qt, exsB, sinkB))
                            workq.append(
                                emit_norm_unit(qi, hB, qt, sinkB, yns))
                            workq.append(emit_transpose_unit(qi, qt, yns))
                            if qt % 2 == 1:
                                og2 = ogp.tile([128, 2 * C], dt.bfloat16,
                                               tag="og", name="og2")
                                rt0 = qi * KPQ + qt - 1
                                for half in range(2):
                                    workq.append(
                                        emit_outproj_unit(psO, rt0 + half,
                                                          og2, half))
                                workq.append(emit_store_unit(og2, rt0))
                        return
                    for h, exs in ((hA, exsA), (hB, exsB)):
                        sink = []
                        for qt in range(KPQ):
                            workq.append(
                                emit_av_unit(psY, qi, h, qt, exs, sink))
                            if qt >= 1:
                                workq.append(
                                    emit_norm_unit(qi, h, qt - 1, sink, yns))
                        workq.append(
                            emit_norm_unit(qi, h, KPQ - 1, sink, yns))

                def emit_transpose_unit(qi, qt, yns):
                    def unit():
                        for c_ in range(2):
                            nc.sync.dma_start_transpose(
                                out=yTn3[:, c_, (qi * KPQ + qt) * 128:
                                         (qi * KPQ + qt + 1) * 128],
                                in_=yns[qt][:, c_ * 128:(c_ + 1) * 128])
                    return unit

                def emit_outproj_unit(psO, rt, og2, half):
                    def unit():
                        for ct in range(2):
                            op = psO.tile([128, 512], dt.float32, tag="op",
                                          name="op")
                            for c_ in range(2):
                                nc.tensor.matmul(
                                    out=op[:],
                                    lhsT=yTn3[:, c_, rt * 128:(rt + 1) * 128],
                                    rhs=wp_sb[:]
                                        .rearrange("p (c e) -> p c e", c=2)
                                        [:, c_, ct * 512:(ct + 1) * 512],
                                    start=(c_ == 0), stop=(c_ == 1))
                            nc.vector.tensor_copy(
                                out=og2[:].rearrange("p (r e) -> p r e", r=2)
                                    [:, half, ct * 512:(ct + 1) * 512],
                                in_=op[:])
                    return unit

                def emit_store_unit(og2, rt0):
                    def unit():
                        nc.sync.dma_start(
                            out=out[rt0 * 128:(rt0 + 2) * 128, :]
                                .rearrange("(r p) e -> p r e", r=2),
                            in_=og2[:].rearrange("p (r e) -> p r e", r=2))
                    return unit

                # m: 0=q(h0,h1) 1=q(h2,h3) 2=k(h0,h1) 3=k(h2,h3)
                dst8 = [stg.tile([128, T], dt.float8e4, tag=f"d8{m}",
                                 name=f"d8{m}", bufs=1) for m in range(4)]

                with (
                    tc.tile_pool(name="psS", bufs=2, space="PSUM",
                                 side="left") as psS,
                    tc.tile_pool(name="psY", bufs=2, space="PSUM",
                                 side="left") as psY,
                ):
                    with tc.tile_pool(name="psPJ", bufs=2, space="PSUM",
                                      side="right") as psPJ:
                        # PE p-state warmup: keep PE busy through the DMA
                        # preamble so the first real chunks run at full clock
                        warm = stg.tile([128, 128], dt.bfloat16, tag="warm",
                                        name="warm", bufs=1)
                        nc.gpsimd.memset(warm[:], 0.0)
                        wps = psPJ.tile([128, 512], dt.float32, tag="pj",
                                        name="wps")
                        for i_ in range(30):
                            nc.tensor.matmul(out=wps[:, 0:128], lhsT=warm[:],
                                             rhs=warm[:], start=(i_ == 0),
                                             stop=(i_ == 29))
                        # q,k halves for heads 0/1 upfront (enough for qi0)
                        for m in (0, 2):
                            for quarter in (0, 1):
                                emit_qk_chunk(psPJ, m, quarter)
                            emit_fold(m, 0)
                        # second halves of m0/m2 as fillers for block (0,0,1)
                        for m in (0, 2):
                            for quarter in (2, 3):
                                projq.append(
                                    (lambda mm, qq:
                                     lambda: emit_qk_chunk(psPJ, mm, qq))
                                    (m, quarter))
                            projq.append(
                                (lambda mm: lambda: emit_fold(mm, 1))(m))

                        yns_all = {}
                        for qi in range(NQI):
                            yns_all[qi] = [
                                ynp.tile([128, 256], dt.bfloat16,
                                         tag=f"yn{q}", name=f"yn{q}")
                                for q in range(KPQ)]

                        def emit_qi_tail(qi):
                            for qt in range(KPQ):
                                workq.append(
                                    emit_transpose_unit(qi, qt, yns_all[qi]))
                            for rp in range(KPQ // 2):
                                rt0 = qi * KPQ + rp * 2
                                og2 = ogp.tile([128, 2 * C], dt.bfloat16,
                                               tag="og", name="og2")
                                for half in range(2):
                                    workq.append(
                                        emit_outproj_unit(psO, rt0 + half,
                                                          og2, half))
                                workq.append(emit_store_unit(og2, rt0))

                        emit_block2(psS, psY, 0, 0, 1, yns_all[0])
                        drain_proj()
                        def qk_split_units(mm, qq):
                            hold = {}

                            def unit_a():
                                hold["xp"] = emit_qk_chunk(
                                    psPJ, mm, qq, Ps=(0, 1))

                            def unit_b():
                                emit_qk_chunk(psPJ, mm, qq, Ps=(2, 3),
                                              xp=hold["xp"])
                            return unit_a, unit_b

                        for m in (1, 3):
                            for quarter in range(4):
                                ua, ub = qk_split_units(m, quarter)
                                projq.append(ua)
                                projq.append(ub)
                            for half in range(2):
                                projq.append(
                                    (lambda mm, hh:
                                     lambda: emit_fold(mm, hh))(m, half))
                        for rt in range(NKT):
                            projq.append(
                                (lambda r: lambda: emit_v_unit(psPJ, r)())(rt))
                        emit_block2(psS, psY, 1, 0, 1, yns_all[1])
                        drain_proj()
                    with tc.tile_pool(name="psO", bufs=2, space="PSUM",
                                      side="right") as psO:
                        emit_block2(psS, psY, 0, 2, 3, yns_all[0])
                        emit_qi_tail(0)
                        emit_block2(psS, psY, 1, 2, 3, yns_all[1])
                        emit_qi_tail(1)
                        while projq or workq:
                            pop()
    nc.compile()
    return nc


def get_nc():
    if "nc" not in _CACHE:
        _CACHE["nc"] = _build_nc()
    return _CACHE["nc"]


def _hilo(a):
    hi = a.astype(FP8)
    lo = (a - hi.astype(np.float32)).astype(FP8)
    return hi, lo


def make_in_maps(x, w_attn, w_proj, freqs_cos, freqs_sin):
    x = np.asarray(x, dtype=np.float32)
    w_attn = np.asarray(w_attn, dtype=np.float32)
    w_proj = np.asarray(w_proj, dtype=np.float32)
    freqs_cos = np.asarray(freqs_cos, dtype=np.float32)
    freqs_sin = np.asarray(freqs_sin, dtype=np.float32)

    # rope tables: per 64-d head block = [32 even | 32 odd], 2 heads/chunk
    cos_t = freqs_cos.T
    sin_t = freqs_sin.T
    cc = np.concatenate([cos_t] * 4, axis=0).astype(BF16)
    ss = np.concatenate([-sin_t, sin_t, -sin_t, sin_t], axis=0)\
        .astype(np.float32)

    pswp = np.zeros((128, 128), dtype=np.float32)
    for i in range(128):
        pswp[i, (i // 32 ^ 1) * 32 + i % 32] = 1.0
    pswp = pswp.astype(BF16)

    atri = np.triu(np.ones((128, 128), dtype=np.float32), k=1).astype(BF16)
    bdg = (NEG * np.eye(128, dtype=np.float32)).astype(BF16)

    perm = np.concatenate([np.arange(0, Dh, 2), np.arange(1, Dh, 2)])

    in_maps = []
    for c in range(NCORES):
        b = c // 4
        h0 = HPC * (c % 4)
        # x^T for this batch: [1024 cin, T], hi/lo, [128, P, s, hl, T]
        xt = np.ascontiguousarray(x[b].reshape(T, C).T)
        xhi, xlo = _hilo(xt)
        x5 = np.stack([xhi.reshape(4, 2, 128, T), xlo.reshape(4, 2, 128, T)],
                      axis=2)                       # [P, s, hl, 128, T]
        xt8 = np.ascontiguousarray(x5.transpose(3, 0, 1, 2, 4)
                                   .reshape(128, -1))

        # wqk columns: m-chunks (q h0h1, q h2h3, k h0h1, k h2h3), each
        # 128 cols = 2 heads x [32 even | 32 odd]
        cols = []
        for off in (0, C):
            for j0 in (0, 2):
                blk = [off + (h0 + j0 + j) * Dh + perm for j in range(2)]
                cols.append(np.concatenate(blk))
        wqk_c = w_attn[:, np.stack(cols, 0).reshape(-1)] * WS  # [1024, 512]
        whi, wlo = _hilo(wqk_c)
        # [P, s, 128, hl, m, 128] -> [128, P, s, hl, m, 128]
        w6 = np.stack([whi.reshape(4, 2, 128, 4, 128),
                       wlo.reshape(4, 2, 128, 4, 128)], axis=3)
        wqk8 = np.ascontiguousarray(w6.transpose(2, 0, 1, 3, 4, 5)
                                    .reshape(128, -1))

        wv_c = w_attn[:, 2 * C + h0 * Dh: 2 * C + (h0 + HPC) * Dh] * WS
        vhi, vlo = _hilo(wv_c)
        v5 = np.stack([vhi.reshape(4, 2, 128, 256),
                       vlo.reshape(4, 2, 128, 256)], axis=3)
        wv8 = np.ascontiguousarray(v5.transpose(2, 0, 1, 3, 4)
                                   .reshape(128, -1))

        wp_c = w_proj[h0 * Dh:(h0 + HPC) * Dh, :]      # [256, 1024]
        wp8 = np.ascontiguousarray(
            wp_c.reshape(2, 128, C).transpose(1, 0, 2).reshape(128, -1))\
            .astype(BF16)

        in_maps.append({
            "xt8": xt8, "wqk8": wqk8, "wv8": wv8, "wp": wp8,
            "cc": cc, "ss": ss, "pswp": pswp, "atri": atri, "bdg": bdg,
        })
    return in_maps


def kernel(x, w_attn, w_proj, freqs_cos, freqs_sin):
    from concourse import bass_utils

    nc = get_nc()
    in_maps = make_in_maps(x, w_attn, w_proj, freqs_cos, freqs_sin)
    res = bass_utils.run_bass_kernel_spmd(
        nc, in_maps, core_ids=list(range(NCORES)), trace=False)
    outs = []
    for b in range(B):
        acc = res.results[4 * b]["out"].astype(np.float32)
        for j in range(1, 4):
            acc += res.results[4 * b + j]["out"].astype(np.float32)
        outs.append(acc)
    return np.stack(outs, 0)


# revision 47
# speedup vs baseline: 1.0069x; 1.0021x over previous
"""Causal self-attention (B=2,T=2048,C=1024,H=16) on 8 trn2 NeuronCores.

Sharding: core c handles batch c//4 and the 4 heads 4*(c%4)..4*(c%4)+3
(head+batch parallel). Each core computes a [2048, 1024] partial of the
output projection (bf16, contraction over its 256 y-dims); host sums the
4 partials per batch in fp32.

Compute strategy:
- qkv projections: fp8e4 DoubleRow matmuls with exact-ish hi+lo fp8
  decomposition of both x and w (hi = fp8(v), lo = fp8(v - hi)).
- rope: PE 32-block-swap matmul + DVE mul/mul/add, writing q/k as fp8
  directly; a pure-layout DMA folds [128(2h x 32e|32o), T] into
  per-head [32, 2, T] fp8 tiles for DoubleRow scores.
- scores: fp8 DoubleRow over the two 32-dim contraction subtiles;
  additive causal mask via bf16 atri/bdg rank trick; exp on Act engine
  into fp8 pair-tiles [128, 2, 1024].
- AV: transposed accumulation out[q,d] with ex as stationary: DoubleRow
  over key-tile pairs, one pass for v_hi and one for v_lo; denominator
  rides as v column 64 (ones in hi, zeros in lo).
- normalize: per-partition reciprocal + tensor_scalar, y in natural
  [q, d] layout; XBAR DMA-transpose to [d, q] bf16 for the out-proj.
- out-proj: bf16, fused og copies + batched stores.

Self-contained: hardcodes all shapes; no sibling imports.
"""
import sys

for _p in ("/opt/trn_rl_repo", "/root/.axon_site/_ro/trn_rl_repo"):
    if _p not in sys.path:
        sys.path.append(_p)

import numpy as np
import ml_dtypes

B, T, C, H = 2, 2048, 1024, 16
Dh = C // H          # 64
NCORES = 8
HPC = 4              # heads per core
NKT = T // 128       # 16 k-tiles
QW = 1024            # q-block width for scores
NQI = T // QW        # 2 q-blocks
KPQ = QW // 128      # 8 k-tiles / q-subtiles per q-block
WS = 32.0            # w_attn pre-scale so fp8 hi/lo avoids subnormals
SCALE = 1.0 / float(np.sqrt(Dh)) / (WS * WS)   # exp scale (q,k carry WS each)
NEG = -30000.0 * WS * WS                       # additive mask, pre-exp-scale

BF16 = ml_dtypes.bfloat16
FP8 = ml_dtypes.float8_e4m3

_CACHE = {}


def _build_nc():
    import concourse.mybir as mybir
    import concourse.tile as tile
    from concourse import bacc

    dt = mybir.dt
    nc = bacc.Bacc("TRN2", target_bir_lowering=False, debug=False,
                   num_devices=NCORES)

    # host layouts (see make_in_maps):
    # xt8:  [128, P(4), s(2), hl(2), T]  fp8  (cin = 256P + 128s + part)
    # wqk8: [128, P, s, hl, m(4), 128]   fp8  (m: q01,q23,k01,k23 dims)
    # wv8:  [128, P, s, hl, 256]         fp8
    # wp:   [128, c(2), 1024]            bf16
    xt8 = nc.dram_tensor("xt8", [128, 4 * 2 * 2 * T], dt.float8e4,
                         kind="ExternalInput").ap()
    wqk8 = nc.dram_tensor("wqk8", [128, 4 * 2 * 2 * 4 * 128], dt.float8e4,
                          kind="ExternalInput").ap()
    wv8 = nc.dram_tensor("wv8", [128, 4 * 2 * 2 * 256], dt.float8e4,
                         kind="ExternalInput").ap()
    wp = nc.dram_tensor("wp", [128, 2 * C], dt.bfloat16,
                        kind="ExternalInput").ap()
    cc = nc.dram_tensor("cc", [128, T], dt.bfloat16, kind="ExternalInput").ap()
    ss = nc.dram_tensor("ss", [128, T], dt.float32, kind="ExternalInput").ap()
    pswp = nc.dram_tensor("pswp", [128, 128], dt.bfloat16,
                          kind="ExternalInput").ap()
    atri = nc.dram_tensor("atri", [128, 128], dt.bfloat16,
                          kind="ExternalInput").ap()
    bdg = nc.dram_tensor("bdg", [128, 128], dt.bfloat16,
                         kind="ExternalInput").ap()
    out = nc.dram_tensor("out", [T, C], dt.bfloat16, kind="ExternalOutput").ap()

    EXP = mybir.ActivationFunctionType.Exp
    DR = mybir.MatmulPerfMode.DoubleRow

    with tile.TileContext(nc) as tc:
        with (
            tc.tile_pool(name="const", bufs=1) as constp,
            tc.tile_pool(name="qk8", bufs=1) as qk8p,
            tc.tile_pool(name="vp8", bufs=1) as vsbp,
            tc.tile_pool(name="exp", bufs=32) as expp,
            tc.tile_pool(name="ynp", bufs=2) as ynp,
            tc.tile_pool(name="ytn", bufs=1) as ytnp,
            tc.tile_pool(name="ogp", bufs=2) as ogp,
            tc.tile_pool(name="dnp", bufs=4) as dnp,
        ):
            # ---------------- constants ----------------
            wqk_sb = constp.tile([128, 4 * 2 * 2 * 4 * 128], dt.float8e4,
                                 tag="wqk", name="wqk_sb")
            wqk_v = wqk_sb[:].rearrange("p (b m e) -> p b m e", b=16, m=4)
            wqk8_v = wqk8.rearrange("p (b m e) -> p b m e", b=16, m=4)
            for m_ in (0, 2):
                nc.sync.dma_start(out=wqk_v[:, :, m_], in_=wqk8_v[:, :, m_])
            wv_sb = constp.tile([128, 4 * 2 * 2 * 256], dt.float8e4,
                                tag="wv", name="wv_sb")
            nc.sync.dma_start(out=wv_sb[:], in_=wv8)
            wp_sb = constp.tile([128, 2 * C], dt.bfloat16, tag="wp",
                                name="wp_sb")
            nc.sync.dma_start(out=wp_sb[:], in_=wp)
            cc_sb = constp.tile([128, T], dt.bfloat16, tag="cc", name="cc_sb")
            nc.sync.dma_start(out=cc_sb[:], in_=cc)
            ss_sb = constp.tile([128, T], dt.float32, tag="ss", name="ss_sb")
            nc.sync.dma_start(out=ss_sb[:], in_=ss)
            pswp_sb = constp.tile([128, 128], dt.bfloat16, tag="pswp",
                                  name="pswp_sb")
            nc.sync.dma_start(out=pswp_sb[:], in_=pswp)
            atri_sb = constp.tile([128, 128], dt.bfloat16, tag="atri",
                                  name="atri_sb")
            nc.sync.dma_start(out=atri_sb[:], in_=atri)
            bdg_sb = constp.tile([128, 128], dt.bfloat16, tag="bdg",
                                 name="bdg_sb")
            nc.sync.dma_start(out=bdg_sb[:], in_=bdg)

            # persistent fp8 q/k per-head tiles [32, 2, T] and v store
            qf = [qk8p.tile([32, 2 * T], dt.float8e4, tag=f"qf{h}",
                            name=f"qf{h}") for h in range(HPC)]
            kf = [qk8p.tile([32, 2 * T], dt.float8e4, tag=f"kf{h}",
                            name=f"kf{h}") for h in range(HPC)]
            # vsb: [128, t(16), hl(2), h(4), 65] fp8
            vsb = vsbp.tile([128, NKT * 2 * HPC * 65], dt.float8e4,
                            tag="vsb", name="vsb")
            vsb4 = vsb[:].rearrange("p (t l h e) -> p t l h e",
                                    t=NKT, l=2, h=HPC)
            # ones col 64: hi=1, lo=0
            nc.gpsimd.memset(vsb[:], 1.0)
            nc.gpsimd.memset(vsb4[:, :, 1, :, 64:65], 0.0)

            # y transposed store [128, c(2), T] bf16 for out-proj
            yTn = ytnp.tile([128, 2 * T], dt.bfloat16, tag="yTn", name="yTn")
            yTn3 = yTn[:].rearrange("p (c t) -> p c t", c=2)

            with tc.tile_pool(name="xtp", bufs=1) as xtp, \
                 tc.tile_pool(name="stg", bufs=3) as stg:
                # xt pair tiles [128, s, hl, T], split-loaded per (P, half)
                xt = [xtp.tile([128, 2 * 2 * T], dt.float8e4, tag=f"xt{P}",
                               name=f"xt{P}") for P in range(4)]
                CH = 2 * 2 * T
                for Th in range(2):
                    for qq_ in range(2):
                        for P in range(4):
                            qc = Th * 1024 + qq_ * 512
                            nc.sync.dma_start(
                                out=xt[P][:].rearrange(
                                    "p (c t) -> p c t", c=4)[:, :, qc:qc + 512],
                                in_=xt8[:, P * CH:(P + 1) * CH].rearrange(
                                    "p (c t) -> p c t", c=4)[:, :, qc:qc + 512])
                    if Th == 0:
                        for m_ in (1, 3):
                            nc.sync.dma_start(out=wqk_v[:, :, m_],
                                              in_=wqk8_v[:, :, m_])
                xt4 = [x[:].rearrange("p (s l t) -> p s l t", s=2, l=2)
                       for x in xt]
                wqk6 = wqk_sb[:].rearrange("p (P s l m e) -> p P s l m e",
                                           P=4, s=2, l=2, m=4)
                wv5 = wv_sb[:].rearrange("p (P s l e) -> p P s l e",
                                         P=4, s=2, l=2)

                qf3 = [q[:].rearrange("p (s t) -> p s t", s=2) for q in qf]
                kf3 = [k[:].rearrange("p (s t) -> p s t", s=2) for k in kf]
                vsb4a = vsb[:].rearrange("p (t l h e) -> p t l h e",
                                         t=NKT, l=2, h=HPC)

                projq = []
                workq = []

                def pop(n=1):
                    for _ in range(n):
                        if projq:
                            projq.pop(0)()
                        elif workq:
                            workq.pop(0)()

                def drain_proj():
                    while projq:
                        projq.pop(0)()

                def emit_qk_chunk(psPJ, m, quarter, Ps=range(4), xp=None):
                    cs = slice(quarter * 512, (quarter + 1) * 512)
                    if xp is None:
                        xp = psPJ.tile([128, 512], dt.float32, tag="pj",
                                       name="xp")
                    for P in Ps:
                        for i, (xl, wl) in enumerate(((0, 0), (1, 0), (0, 1))):
                            nc.tensor.matmul(
                                out=xp[:],
                                lhsT=wqk6[:, P, :, wl, m],
                                rhs=xt4[P][:, :, xl, cs],
                                start=(P == 0 and i == 0),
                                stop=(P == 3 and i == 2),
                                perf_mode=DR)
                    if max(Ps) < 3:
                        return xp
                    xsb = stg.tile([128, 512], dt.bfloat16, tag="xsb",
                                   name="xsb")
                    nc.scalar.copy(out=xsb[:], in_=xp[:])
                    xs = psPJ.tile([128, 512], dt.float32, tag="pj", name="xs")
                    nc.tensor.matmul(out=xs[:], lhsT=pswp_sb[:], rhs=xsb[:],
                                     start=True, stop=True)
                    r1 = stg.tile([128, 512], dt.bfloat16, tag="r1", name="r1")
                    nc.vector.tensor_mul(out=r1[:], in0=xsb[:], in1=cc_sb[:, cs])
                    r2 = stg.tile([128, 512], dt.bfloat16, tag="r2", name="r2")
                    nc.vector.tensor_mul(out=r2[:], in0=xs[:], in1=ss_sb[:, cs])
                    nc.vector.tensor_add(out=dst8[m][:, cs], in0=r1[:],
                                         in1=r2[:])

                def emit_fold(m, half):
                    cs = slice(half * 1024, half * 1024 + 1024)
                    for j in range(2):
                        dest = (qf if m < 2 else kf)[(m % 2) * 2 + j]
                        for s_ in range(2):
                            nc.sync.dma_start(
                                out=dest[:, s_ * T + half * 1024:
                                         s_ * T + half * 1024 + 1024],
                                in_=dst8[m][j * 64 + s_ * 32:
                                            j * 64 + (s_ + 1) * 32, cs])

                def emit_v_unit(psPJ, rt):
                    def unit():
                        vp = psPJ.tile([128, 256], dt.float32, tag="pj",
                                       name="vp")
                        ts = slice(rt * 128, (rt + 1) * 128)
                        for P in range(4):
                            for i, (xl, wl) in enumerate(((0, 0), (1, 0),
                                                          (0, 1))):
                                nc.tensor.matmul(
                                    out=vp[:],
                                    lhsT=xt4[P][:, :, xl, ts],
                                    rhs=wv5[:, P, :, wl],
                                    start=(P == 0 and i == 0),
                                    stop=(P == 3 and i == 2),
                                    perf_mode=DR)
                        vp3 = vp[:].rearrange("p (h e) -> p h e", h=HPC)
                        nc.vector.tensor_copy(out=vsb4a[:, rt, 0, :, 0:64],
                                              in_=vp3)
                        nc.vector.tensor_sub(out=vsb4a[:, rt, 1, :, 0:64],
                                             in0=vp3,
                                             in1=vsb4a[:, rt, 0, :, 0:64])
                    return unit

                def emit_scores(psS, h, qi, t):
                    p = t - KPQ * qi
                    j0 = 128 * p if p > 0 else 0
                    mask_bank = j0 // 512 if p >= 0 else -1
                    sc = psS.tile([128, QW], dt.float32, tag="sc", name="sc")
                    for bk in range(2):
                        lo, hi = bk * 512, (bk + 1) * 512
                        lo = max(lo, j0)
                        if lo >= hi:
                            continue
                        nc.tensor.matmul(
                            out=sc[:, lo:hi],
                            lhsT=kf3[h][:, :, t * 128:(t + 1) * 128],
                            rhs=qf3[h][:, :, qi * QW + lo:qi * QW + hi],
                            start=True, stop=(bk != mask_bank),
                            perf_mode=DR)
                    if p >= 0:
                        nc.tensor.matmul(
                            out=sc[:, j0:j0 + 128],
                            lhsT=atri_sb[:], rhs=bdg_sb[:],
                            start=False, stop=True)
                    return sc, j0

                def emit_av_unit(psY, qi, h, qt, exs, sink):
                    def unit():
                        qtg = qi * KPQ + qt
                        cs = slice(qt * 128, (qt + 1) * 128)
                        yt = psY.tile([128, 65], dt.float32, tag="yt",
                                      name="yt")
                        nfull = (qtg + 1) // 2
                        single = (qtg % 2 == 0)
                        for u in range(nfull):
                            ex3 = exs[u][:].rearrange("p (s q) -> p s q", s=2)
                            for li in range(2):
                                is_last = (not single and u == nfull - 1
                                           and li == 1)
                                nc.tensor.matmul(
                                    out=yt[:],
                                    lhsT=ex3[:, :, cs],
                                    rhs=vsb4a[:, 2 * u:2 * u + 2, li, h],
                                    start=(u == 0 and li == 0), stop=is_last,
                                    perf_mode=DR)
                        if single:
                            ts_ = qtg
                            ex2 = exs[ts_ // 2][:].rearrange(
                                "p (s q) -> p s q", s=2)[:, ts_ % 2]
                            for li in range(2):
                                nc.tensor.matmul(
                                    out=yt[:],
                                    lhsT=ex2[:, cs],
                                    rhs=vsb4a[:, ts_, li, h],
                                    start=(qtg == 0 and li == 0),
                                    stop=(li == 1))
                        sink.append(yt)
                    return unit

                def emit_norm_unit(qi, h, qt, sink, yns):
                    def unit():
                        yt = sink.pop(0)
                        dn = dnp.tile([128, 1], dt.float32, tag="dn",
                                      name="dn")
                        nc.vector.reciprocal(out=dn[:], in_=yt[:, 64:65])
                        nc.vector.tensor_scalar(
                            out=yns[qt][:, h * 64:(h + 1) * 64],
                            in0=yt[:, 0:64],
                            scalar1=dn[:], scalar2=1.0 / WS,
                            op0=mybir.AluOpType.mult,
                            op1=mybir.AluOpType.mult)
                    return unit

                def emit_block2(psS, psY, qi, hA, hB, yns, flush=None,
                                chain=False):
                    tmax = KPQ * qi + KPQ - 1
                    exsA, exsB = [], []
                    sinkA, sinkB = [], []

                    def flush_qt(qt):
                        emit_av_unit(psY, qi, hA, qt, exsA, sinkA)()
                        emit_norm_unit(qi, hA, qt, sinkA, yns)()
                        emit_av_unit(psY, qi, hB, qt, exsB, sinkB)()
                        emit_norm_unit(qi, hB, qt, sinkB, yns)()
                        if flush is not None:
                            flush(qt)

                    for t in range(tmax + 1):
                        scA, j0 = emit_scores(psS, hA, qi, t)
                        scB, _ = emit_scores(psS, hB, qi, t)
                        if t % 2 == 0:
                            exsA.append(expp.tile([128, 2 * QW], dt.float8e4,
                                                  tag="ex", name="exA"))
                            exsB.append(expp.tile([128, 2 * QW], dt.float8e4,
                                                  tag="ex", name="exB"))
                        nc.scalar.activation(
                            out=exsA[-1][:, (t % 2) * QW + j0:
                                         (t % 2) * QW + QW],
                            in_=scA[:, j0:QW], func=EXP, scale=SCALE)
                        nc.scalar.activation(
                            out=exsB[-1][:, (t % 2) * QW + j0:
                                         (t % 2) * QW + QW],
                            in_=scB[:, j0:QW], func=EXP, scale=SCALE)
                        if flush is not None:
                            kq = t - (KPQ * qi) - 3
                            if 0 <= kq < KPQ:
                                flush_qt(kq)
                            pop(1)
                        else:
                            pop(2)
                    if flush is not None:
                        for kq in range(max(0, tmax - KPQ * qi - 2), KPQ):
                            flush_qt(kq)
                        return
                    if chain:
                        sinkA, sinkB = [], []
                        for qt in range(KPQ):
                            workq.append(
                                emit_av_unit(psY, qi, hA, qt, exsA, sinkA))
                            workq.append(
                                emit_norm_unit(qi, hA, qt, sinkA, yns))
                            workq.append(
                                emit_av_unit(psY, qi, hB, qt, exsB, sinkB))
                            workq.append(
                                emit_norm_unit(qi, hB, qt, sinkB, yns))
                            workq.append(emit_transpose_unit(qi, qt, yns))
                            if qt % 2 == 1:
                                og2 = ogp.tile([128, 2 * C], dt.bfloat16,
                                               tag="og", name="og2")
                                rt0 = qi * KPQ + qt - 1
                                for half in range(2):
                                    workq.append(
                                        emit_outproj_unit(psO, rt0 + half,
                                                          og2, half))
                                workq.append(emit_store_unit(og2, rt0))
                        return
                    for h, exs in ((hA, exsA), (hB, exsB)):
                        sink = []
                        for qt in range(KPQ):
                            workq.append(
                                emit_av_unit(psY, qi, h, qt, exs, sink))
                            if qt >= 1:
                                workq.append(
                                    emit_norm_unit(qi, h, qt - 1, sink, yns))
                        workq.append(
                            emit_norm_unit(qi, h, KPQ - 1, sink, yns))

                def emit_transpose_unit(qi, qt, yns):
                    def unit():
                        for c_ in range(2):
                            nc.sync.dma_start_transpose(
                                out=yTn3[:, c_, (qi * KPQ + qt) * 128:
                                         (qi * KPQ + qt + 1) * 128],
                                in_=yns[qt][:, c_ * 128:(c_ + 1) * 128])
                    return unit

                def emit_outproj_unit(psO, rt, og2, half):
                    def unit():
                        for ct in range(2):
                            op = psO.tile([128, 512], dt.float32, tag="op",
                                          name="op")
                            for c_ in range(2):
                                nc.tensor.matmul(
                                    out=op[:],
                                    lhsT=yTn3[:, c_, rt * 128:(rt + 1) * 128],
                                    rhs=wp_sb[:]
                                        .rearrange("p (c e) -> p c e", c=2)
                                        [:, c_, ct * 512:(ct + 1) * 512],
                                    start=(c_ == 0), stop=(c_ == 1))
                            nc.vector.tensor_copy(
                                out=og2[:].rearrange("p (r e) -> p r e", r=2)
                                    [:, half, ct * 512:(ct + 1) * 512],
                                in_=op[:])
                    return unit

                def emit_store_unit(og2, rt0):
                    def unit():
                        nc.sync.dma_start(
                            out=out[rt0 * 128:(rt0 + 2) * 128, :]
                                .rearrange("(r p) e -> p r e", r=2),
                            in_=og2[:].rearrange("p (r e) -> p r e", r=2))
                    return unit

                # m: 0=q(h0,h1) 1=q(h2,h3) 2=k(h0,h1) 3=k(h2,h3)
                dst8 = [stg.tile([128, T], dt.float8e4, tag=f"d8{m}",
                                 name=f"d8{m}", bufs=1) for m in range(4)]

                with (
                    tc.tile_pool(name="psS", bufs=2, space="PSUM",
                                 side="left") as psS,
                    tc.tile_pool(name="psY", bufs=2, space="PSUM",
                                 side="left") as psY,
                ):
                    with tc.tile_pool(name="psPJ", bufs=2, space="PSUM",
                                      side="right") as psPJ:
                        # PE p-state warmup: keep PE busy through the DMA
                        # preamble so the first real chunks run at full clock
                        warm = stg.tile([128, 128], dt.bfloat16, tag="warm",
                                        name="warm", bufs=1)
                        nc.gpsimd.memset(warm[:], 0.0)
                        def warm_n(n):
                            wy = psY.tile([128, 65], dt.float32, tag="yt",
                                          name="wy")
                            for i_ in range(n):
                                nc.tensor.matmul(
                                    out=wy[0:64, 0:64],
                                    lhsT=warm[:, 0:64], rhs=warm[:, 0:64],
                                    start=(i_ == 0), stop=(i_ == n - 1))

                        wps = psPJ.tile([128, 512], dt.float32, tag="pj",
                                        name="wps")
                        for i_ in range(30):
                            nc.tensor.matmul(out=wps[:, 0:128], lhsT=warm[:],
                                             rhs=warm[:], start=(i_ == 0),
                                             stop=(i_ == 29))
                        # q,k halves for heads 0/1 upfront (enough for qi0),
                        # warmup delay-slots between chunks keep PE at full
                        # p-state through the DMA waits
                        for m in (0, 2):
                            for quarter in (0, 1):
                                emit_qk_chunk(psPJ, m, quarter)
                                warm_n(12)
                            emit_fold(m, 0)
                        # second halves of m0/m2 as fillers for block (0,0,1)
                        for m in (0, 2):
                            for quarter in (2, 3):
                                projq.append(
                                    (lambda mm, qq:
                                     lambda: emit_qk_chunk(psPJ, mm, qq))
                                    (m, quarter))
                            projq.append(
                                (lambda mm: lambda: emit_fold(mm, 1))(m))

                        yns_all = {}
                        for qi in range(NQI):
                            yns_all[qi] = [
                                ynp.tile([128, 256], dt.bfloat16,
                                         tag=f"yn{q}", name=f"yn{q}")
                                for q in range(KPQ)]

                        def emit_qi_tail(qi):
                            for qt in range(KPQ):
                                workq.append(
                                    emit_transpose_unit(qi, qt, yns_all[qi]))
                            for rp in range(KPQ // 2):
                                rt0 = qi * KPQ + rp * 2
                                og2 = ogp.tile([128, 2 * C], dt.bfloat16,
                                               tag="og", name="og2")
                                for half in range(2):
                                    workq.append(
                                        emit_outproj_unit(psO, rt0 + half,
                                                          og2, half))
                                workq.append(emit_store_unit(og2, rt0))

                        emit_block2(psS, psY, 0, 0, 1, yns_all[0])
                        drain_proj()
                        def qk_split_units(mm, qq):
                            hold = {}

                            def unit_a():
                                hold["xp"] = emit_qk_chunk(
                                    psPJ, mm, qq, Ps=(0, 1))

                            def unit_b():
                                emit_qk_chunk(psPJ, mm, qq, Ps=(2, 3),
                                              xp=hold["xp"])
                            return unit_a, unit_b

                        for m in (1, 3):
                            for quarter in range(4):
                                ua, ub = qk_split_units(m, quarter)
                                projq.append(ua)
                                projq.append(ub)
                            for half in range(2):
                                projq.append(
                                    (lambda mm, hh:
                                     lambda: emit_fold(mm, hh))(m, half))
                        for rt in range(NKT):
                            projq.append(
                                (lambda r: lambda: emit_v_unit(psPJ, r)())(rt))
                        emit_block2(psS, psY, 1, 0, 1, yns_all[1])
                        drain_proj()
                    with tc.tile_pool(name="psO", bufs=2, space="PSUM",
                                      side="right") as psO:
                        emit_block2(psS, psY, 0, 2, 3, yns_all[0])
                        emit_qi_tail(0)
                        emit_block2(psS, psY, 1, 2, 3, yns_all[1])
                        emit_qi_tail(1)
                        while projq or workq:
                            pop()
    nc.compile()
    return nc


def get_nc():
    if "nc" not in _CACHE:
        _CACHE["nc"] = _build_nc()
    return _CACHE["nc"]


def _hilo(a):
    hi = a.astype(FP8)
    lo = (a - hi.astype(np.float32)).astype(FP8)
    return hi, lo


def make_in_maps(x, w_attn, w_proj, freqs_cos, freqs_sin):
    x = np.asarray(x, dtype=np.float32)
    w_attn = np.asarray(w_attn, dtype=np.float32)
    w_proj = np.asarray(w_proj, dtype=np.float32)
    freqs_cos = np.asarray(freqs_cos, dtype=np.float32)
    freqs_sin = np.asarray(freqs_sin, dtype=np.float32)

    # rope tables: per 64-d head block = [32 even | 32 odd], 2 heads/chunk
    cos_t = freqs_cos.T
    sin_t = freqs_sin.T
    cc = np.concatenate([cos_t] * 4, axis=0).astype(BF16)
    ss = np.concatenate([-sin_t, sin_t, -sin_t, sin_t], axis=0)\
        .astype(np.float32)

    pswp = np.zeros((128, 128), dtype=np.float32)
    for i in range(128):
        pswp[i, (i // 32 ^ 1) * 32 + i % 32] = 1.0
    pswp = pswp.astype(BF16)

    atri = np.triu(np.ones((128, 128), dtype=np.float32), k=1).astype(BF16)
    bdg = (NEG * np.eye(128, dtype=np.float32)).astype(BF16)

    perm = np.concatenate([np.arange(0, Dh, 2), np.arange(1, Dh, 2)])

    in_maps = []
    for c in range(NCORES):
        b = c // 4
        h0 = HPC * (c % 4)
        # x^T for this batch: [1024 cin, T], hi/lo, [128, P, s, hl, T]
        xt = np.ascontiguousarray(x[b].reshape(T, C).T)
        xhi, xlo = _hilo(xt)
        x5 = np.stack([xhi.reshape(4, 2, 128, T), xlo.reshape(4, 2, 128, T)],
                      axis=2)                       # [P, s, hl, 128, T]
        xt8 = np.ascontiguousarray(x5.transpose(3, 0, 1, 2, 4)
                                   .reshape(128, -1))

        # wqk columns: m-chunks (q h0h1, q h2h3, k h0h1, k h2h3), each
        # 128 cols = 2 heads x [32 even | 32 odd]
        cols = []
        for off in (0, C):
            for j0 in (0, 2):
                blk = [off + (h0 + j0 + j) * Dh + perm for j in range(2)]
                cols.append(np.concatenate(blk))
        wqk_c = w_attn[:, np.stack(cols, 0).reshape(-1)] * WS  # [1024, 512]
        whi, wlo = _hilo(wqk_c)
        # [P, s, 128, hl, m, 128] -> [128, P, s, hl, m, 128]
        w6 = np.stack([whi.reshape(4, 2, 128, 4, 128),
                       wlo.reshape(4, 2, 128, 4, 128)], axis=3)
        wqk8 = np.ascontiguousarray(w6.transpose(2, 0, 1, 3, 4, 5)
                                    .reshape(128, -1))

        wv_c = w_attn[:, 2 * C + h0 * Dh: 2 * C + (h0 + HPC) * Dh] * WS
        vhi, vlo = _hilo(wv_c)
        v5 = np.stack([vhi.reshape(4, 2, 128, 256),
                       vlo.reshape(4, 2, 128, 256)], axis=3)
        wv8 = np.ascontiguousarray(v5.transpose(2, 0, 1, 3, 4)
                                   .reshape(128, -1))

        wp_c = w_proj[h0 * Dh:(h0 + HPC) * Dh, :]      # [256, 1024]
        wp8 = np.ascontiguousarray(
            wp_c.reshape(2, 128, C).transpose(1, 0, 2).reshape(128, -1))\
            .astype(BF16)

        in_maps.append({
            "xt8": xt8, "wqk8": wqk8, "wv8": wv8, "wp": wp8,
            "cc": cc, "ss": ss, "pswp": pswp, "atri": atri, "bdg": bdg,
        })
    return in_maps


def kernel(x, w_attn, w_proj, freqs_cos, freqs_sin):
    from concourse import bass_utils

    nc = get_nc()
    in_maps = make_in_maps(x, w_attn, w_proj, freqs_cos, freqs_sin)
    res = bass_utils.run_bass_kernel_spmd(
        nc, in_maps, core_ids=list(range(NCORES)), trace=False)
    outs = []
    for b in range(B):
        acc = res.results[4 * b]["out"].astype(np.float32)
        for j in range(1, 4):
            acc += res.results[4 * b + j]["out"].astype(np.float32)
        outs.append(acc)
    return np.stack(outs, 0)
